# revision 33
# baseline (speedup 1.0000x reference)
"""Bass/Tile kernel for nn_Att_28879360099124 on 8 TRN2 NeuronCores.

Computes, for full inputs
    hiddenState [TQ=1024, B=16, H=1024] f32
    encoderOut  [S=4096,  B=16, H=1024] f32
the reference
    scores = einsum('sbh,tbh->bst')          # [B, S, TQ]
    attW   = softmax(tanh(scores), axis=S)   # [B, S, TQ]

Strategy: data-parallel over B (2 batches per core, no communication).
Per core, per batch b:
  - score tiles are [t_p=128, s_f] so the softmax axis (s) is the free dim.
  - matmul: psum[t128, s512] += hidT[h128, t128].T @ encT[h128, s512],
    accumulated over 8 h-tiles, fp32r inputs (full PE speed, ~TF32 precision).
  - ACT: tanh in-place on psum, then exp psum->SBUF with accum_out giving
    the per-t partial row sum of each s-block for free.
  - DVE: reduce the 8 partials, reciprocal, per-partition scale; out via
    gpsimd (SWDGE) so stores never block input loads on the Sync queue.
encT is SBUF-resident per batch (8 tiles [128, 4096] fp32 = 128KB/partition);
hidT streams per t-tile; exp rows live only for one t-tile.
The first t-tile of each batch and the last t-tile of batch 0 order their
matmuls h-outer: the first chases the arriving enc DMA stream, the last
releases enc tiles early+staggered so the next batch's enc loads overlap
tail compute (avoids a ~45us PE stall and the HAM re-throttle).

Host side: inputs are pre-transposed to [B, H, *] and the output is produced
as [B, TQ, S] then transposed to [B, S, TQ]; only HW time counts.
"""

import numpy as np

TQ, B, H, S = 1024, 16, 1024, 4096
NCORES = 8
B_LOC = B // NCORES  # batches per core
P = 128
HT = H // P          # 8 h-tiles
TT = TQ // P         # 8 t-tiles per batch
SBLK = 512           # matmul moving free dim (fp32 max, one PSUM bank)
NSB = S // SBLK      # 8 s-blocks

_CACHE = {}


def _build():
    import concourse.bacc as bacc
    import concourse.mybir as mybir
    import concourse.tile as tile

    f32 = mybir.dt.float32
    f32r = mybir.dt.float32r
    Act = mybir.ActivationFunctionType

    nc = bacc.Bacc("TRN2", target_bir_lowering=False, debug=False,
                   num_devices=NCORES)

    hid_d = nc.dram_tensor("hidT", [B_LOC, HT, P, TQ], f32r,
                           kind="ExternalInput").ap()
    enc_d = nc.dram_tensor("encT", [B_LOC, HT, P, S], f32r,
                           kind="ExternalInput").ap()
    out_d = nc.dram_tensor("attW", [B_LOC, TT, P, S], f32,
                           kind="ExternalOutput").ap()

    with tile.TileContext(nc) as tc:
        with (
            tc.tile_pool(name="encp", bufs=HT + 2) as encp,
            tc.tile_pool(name="hidp", bufs=3) as hidp,
            tc.tile_pool(name="expp", bufs=2) as expp,
            tc.tile_pool(name="smallp", bufs=4) as smallp,
            tc.tile_pool(name="psum", bufs=4, space="PSUM") as psump,
        ):
            def load_hid(b, ti):
                # stationary weights for this t-tile: [128(h), HT, 128(t)]
                hid_t = hidp.tile([P, HT, P], f32r, name=f"hid_{b}_{ti}",
                                  tag="hid")
                nc.sync.dma_start(
                    out=hid_t,
                    in_=hid_d[b, :, :, ti * P:(ti + 1) * P].rearrange(
                        "hi hp t -> hp hi t"),
                )
                return hid_t

            for b in range(B_LOC):
                # Chase tiles' weights BEFORE the enc tiles: the HWDGE
                # queue is FIFO, so anything behind the 16MB enc load
                # completes last -- and both tiles chase the arriving
                # enc stream.
                hid_pre = {ti: load_hid(b, ti) for ti in range(2)}

                # encoder tiles for this batch: 8 x [128(h), S] fp32r.
                # Loaded in 1MB quarters, quarter-major, so the first
                # t-tiles' matmuls can chase the arrival stream with sub-1us
                # granularity (keeps gaps under the 3.4us HAM re-throttle
                # window).
                enc_tiles = [encp.tile([P, S], f32r, name=f"enc_{b}_{hi}",
                                       tag="enc")
                             for hi in range(HT)]
                Q = S // 4
                for q in range(4):
                    for hi in range(HT):
                        nc.sync.dma_start(
                            out=enc_tiles[hi][:, q * Q:(q + 1) * Q],
                            in_=enc_d[b, hi, :, q * Q:(q + 1) * Q])

                def finalize(ti, exp_row, partials, n_acc, last_tile):
                    sums = smallp.tile([P, 1], f32, name=f"sum_{b}_{ti}",
                                       tag="sums")
                    nc.vector.reduce_sum(out=sums, in_=partials[:, :n_acc],
                                         axis=mybir.AxisListType.X)
                    recip = smallp.tile([P, 1], f32, name=f"rcp_{b}_{ti}",
                                        tag="recip")
                    nc.vector.reciprocal(out=recip, in_=sums)
                    # Stores on gpsimd (SWDGE) so they can't block input
                    # loads on the sync queue -- except the very last tile,
                    # whose stores use the by-then-idle sync queue so the
                    # slow SWDGE drain starts earlier (and go single-block
                    # for a shorter serial tail).
                    dma_eng = nc.sync if last_tile else nc.gpsimd
                    step = 1 if last_tile else 2
                    for sc in range(0, NSB, step):
                        nc.vector.tensor_scalar_mul(
                            exp_row[:, sc:sc + step], exp_row[:, sc:sc + step],
                            recip)
                        dma_eng.dma_start(
                            out=out_d[b, ti, :, sc * SBLK:(sc + step) * SBLK],
                            in_=exp_row[:, sc:sc + step],
                        )

                # ---- fused quarter-major chase over t-tiles 0 and 1 ----
                # Both tiles' matmuls are interleaved per enc quarter so the
                # in-order PE queue always has dense work matching the DMA
                # arrival stream (a single tile only has ~14us of matmuls
                # against a ~40us enc load).
                chase_exp = [expp.tile([P, NSB, SBLK], f32,
                                       name=f"exp_{b}_{j}", tag="exp")
                             for j in range(2)]
                chase_part = [smallp.tile([P, NSB], f32,
                                          name=f"part_{b}_{j}", tag="part")
                              for j in range(2)]
                for q in range(4):
                    tq = [psump.tile([P, 2, SBLK], f32,
                                     name=f"ps_{b}_{j}_{q}", tag="ps")
                          for j in range(2)]
                    for hi in range(HT):
                        for j in range(2):
                            for col in range(2):
                                si = 2 * q + col
                                nc.tensor.matmul(
                                    tq[j][:, col],
                                    lhsT=hid_pre[j][:, hi, :],
                                    rhs=enc_tiles[hi][:, si * SBLK:
                                                      (si + 1) * SBLK],
                                    start=hi == 0,
                                    stop=hi == HT - 1,
                                )
                    for j in range(2):
                        nc.scalar.activation(tq[j], tq[j], Act.Tanh)
                        nc.scalar.activation(
                            chase_exp[j][:, 2 * q:2 * q + 2], tq[j], Act.Exp,
                            accum_out=chase_part[j][:, q:q + 1])
                for j in range(2):
                    finalize(j, chase_exp[j], chase_part[j], 4, False)

                # ---- t-tiles 2..7: steady state ----
                for ti in range(2, TT):
                    hid_t = load_hid(b, ti)

                    exp_row = expp.tile([P, NSB, SBLK], f32,
                                        name=f"exp_{b}_{ti}", tag="exp")

                    # h-outer for the last tile of batch 0 (staggered early
                    # release of enc slots for the next batch's prefetch);
                    # s-outer else.
                    h_outer = ti == TT - 1 and b < B_LOC - 1
                    last_tile = b == B_LOC - 1 and ti == TT - 1

                    # 2-bank psum tiles: ACT runs [128,1024] passes,
                    # amortizing its ~250ns fixed overhead per instruction.
                    pss = [psump.tile([P, 2, SBLK], f32,
                                      name=f"ps_{b}_{ti}_{sp}", tag="ps")
                           for sp in range(NSB // 2)]

                    def mm(si, hi):
                        nc.tensor.matmul(
                            pss[si // 2][:, si % 2],
                            lhsT=hid_t[:, hi, :],
                            rhs=enc_tiles[hi][:, si * SBLK:(si + 1) * SBLK],
                            start=hi == 0,
                            stop=hi == HT - 1,
                        )

                    if h_outer:
                        for hi in range(HT):
                            for si in range(NSB):
                                mm(si, hi)
                    else:
                        for si in range(NSB):
                            for hi in range(HT):
                                mm(si, hi)

                    partials = smallp.tile([P, NSB], f32,
                                           name=f"part_{b}_{ti}", tag="part")
                    if last_tile:
                        # single-bank passes: shorter serial chain after the
                        # final matmul
                        n_acc = NSB
                        for si in range(NSB):
                            blk = pss[si // 2][:, si % 2]
                            nc.scalar.activation(blk, blk, Act.Tanh)
                            nc.scalar.activation(
                                exp_row[:, si], blk, Act.Exp,
                                accum_out=partials[:, si:si + 1])
                    else:
                        n_acc = NSB // 2
                        for sp in range(NSB // 2):
                            # tanh in place on psum, then exp -> SBUF + sums
                            nc.scalar.activation(pss[sp], pss[sp], Act.Tanh)
                            nc.scalar.activation(
                                exp_row[:, 2 * sp:2 * sp + 2], pss[sp], Act.Exp,
                                accum_out=partials[:, sp:sp + 1])

                    finalize(ti, exp_row, partials, n_acc, last_tile)
    nc.compile()
    return nc


def kernel(hiddenState: np.ndarray, encoderOut: np.ndarray) -> np.ndarray:
    from concourse import bass_utils

    hiddenState = np.asarray(hiddenState, dtype=np.float32)
    encoderOut = np.asarray(encoderOut, dtype=np.float32)

    # [TQ, B, H] -> [B, HT, P, TQ]; [S, B, H] -> [B, HT, P, S]
    hidT = np.ascontiguousarray(hiddenState.transpose(1, 2, 0)).reshape(
        B, HT, P, TQ)
    encT = np.ascontiguousarray(encoderOut.transpose(1, 2, 0)).reshape(
        B, HT, P, S)

    if "nc" not in _CACHE:
        _CACHE["nc"] = _build()
    nc = _CACHE["nc"]

    in_maps = [
        {"hidT": hidT[c * B_LOC:(c + 1) * B_LOC],
         "encT": encT[c * B_LOC:(c + 1) * B_LOC]}
        for c in range(NCORES)
    ]
    res = bass_utils.run_bass_kernel_spmd(
        nc, in_maps, core_ids=list(range(NCORES)))
    _CACHE["last_results"] = res

    # per-core [B_LOC, TT, P, S] -> full [B, TQ, S] -> [B, S, TQ]
    out = np.concatenate([r["attW"] for r in res.results], axis=0)
    out = out.reshape(B, TQ, S).transpose(0, 2, 1)
    return np.ascontiguousarray(out)
